# revision 15
# baseline (speedup 1.0000x reference)
"""3-layer GCN (PyG GCNConv-style) on 8 Trainium2 NeuronCores.

Design (dense-stream gather + one-hot PE segment-sum):
- dst nodes LPT-bin-packed into 784 (core,tile) bins of <=128, nodes relabeled
  (core,tile,row)-major; all feature tables stored in relabeled order.
- Per core, edges grouped by (tile j, src-chunk c) with compile-time budgets
  Q[j,c] = max-over-cores count (SPMD program; per-core variation is data).
- 4 src-chunks of 25000 rows (dma_gather int16 index limit), one gather stream
  per chunk on its own SWDGE queue (4 queues), calls of 1024 idx (64-desc/engine
  packet ceiling), 256B rows (L2 native 128-wide bf16; L3 duplicated 64-wide).
- Segment-sum: per 128-slot stream column, a one-hot sel built on DVE
  (dloc==iota) maps slots -> dst rows; PE matmuls accumulate into PSUM z per
  tile; self-loops via identity matmul over the tile's own table rows.
- L1 messages host-expanded (pure gather of x*dinv rows) and streamed with
  plain contiguous HWDGE DMA - no device gather for layer 1.
- Backend: z*dinv -> PE transpose -> GEMM (bias via aug row / K=1 ones matmul)
  -> celu (DVE+ACT) -> next table block; AllGather (internal Shared DRAM)
  between layers.
"""
import numpy as np
import ml_dtypes

bf16 = ml_dtypes.bfloat16

N = 100000
NC = 8
NPC = N // NC             # 12500
P = 128
TILES = 98                # 97*128 + 84
LAST_ROWS = NPC - 97 * P  # 84
NCHUNK = 4
CHUNK = N // NCHUNK       # 25000
CALL = 1024
LAST_EXEC_NS = None
LAST_TRACE = None


def _host_prep(edge_index):
    import heapq
    src0 = edge_index[0].astype(np.int64)
    dst0 = edge_index[1].astype(np.int64)
    deg = np.bincount(dst0, minlength=N).astype(np.float32) + 1.0  # self loop
    dinv = (1.0 / np.sqrt(deg)).astype(np.float32)

    caps = np.full(NC * TILES, P, np.int64)
    caps[TILES - 1::TILES] = LAST_ROWS
    order = np.argsort(-deg, kind="stable")
    heap = [(0.0, b) for b in range(NC * TILES)]
    heapq.heapify(heap)
    members = [[] for _ in range(NC * TILES)]
    for v in order:
        while True:
            load, b = heapq.heappop(heap)
            if len(members[b]) < caps[b]:
                break
        members[b].append(int(v))
        if len(members[b]) < caps[b]:
            heapq.heappush(heap, (load + float(deg[v]), b))
    newid = np.empty(N, np.int64)
    for b in range(NC * TILES):
        c, j = b // TILES, b % TILES
        mem = np.array(members[b], np.int64)
        newid[mem] = c * NPC + j * P + np.arange(len(mem))
    orig_of_new = np.empty(N, np.int64)
    orig_of_new[newid] = np.arange(N)

    sN = newid[src0]
    dN = newid[dst0]
    core = dN // NPC
    loc = dN % NPC
    j_of = loc // P
    row_of = loc - j_of * P
    s2 = (sN // NPC) * B2 + sN % NPC
    ch = s2 // C2

    runkey = ((core * TILES + j_of) * NCHUNK + ch) * P + row_of
    rk, rcnt = np.unique(runkey, return_counts=True)
    rpad = -(-rcnt // G) * G
    r_core = rk // (TILES * NCHUNK * P)
    r_jc = (rk // P) % (TILES * NCHUNK)
    cnt4 = np.zeros((NC, TILES * NCHUNK), np.int64)
    np.add.at(cnt4, (r_core, r_jc), rpad)
    Q4 = cnt4.max(axis=0).reshape(TILES, NCHUNK)
    cum4 = np.zeros((TILES, NCHUNK), np.int64)
    for c4 in range(NCHUNK):
        cum4[:, c4] = np.concatenate([[0], np.cumsum(Q4[:-1, c4])])
    S4 = Q4.sum(axis=0)
    CALLS = [int(-(-S4[c4] // CALL)) for c4 in range(NCHUNK)]
    Spad = [CALLS[c4] * CALL for c4 in range(NCHUNK)]

    entries = []
    for j in range(TILES):
        for c4 in range(NCHUNK):
            a, b2 = int(cum4[j, c4]) // G, int(cum4[j, c4] + Q4[j, c4]) // G
            if b2 == a:
                continue
            for K in range(a // P, -(-b2 // P)):
                entries.append((j, c4, K))
    NENT = len(entries)

    per_core = []
    for c8 in range(NC):
        m = core == c8
        ej, ec, erow, es2 = j_of[m], ch[m], row_of[m], s2[m]
        o = np.lexsort((erow, ec, ej))
        ej, ec, erow, es2 = ej[o], ec[o], erow[o], es2[o]
        rkey = (ej * NCHUNK + ec) * P + erow
        urk, ucnt = np.unique(rkey, return_counts=True)
        ugr = (-(-ucnt // G) * G) // G
        ujc = urk // P
        udrow = urk % P
        gofs_glob = np.cumsum(ugr) - ugr
        jc_first = np.searchsorted(ujc, np.arange(TILES * NCHUNK))
        base_jc = np.zeros(TILES * NCHUNK, np.int64)
        have = jc_first < len(ujc)
        idxf = np.minimum(jc_first, len(ujc) - 1)
        base_jc[have] = gofs_glob[idxf[have]]
        run_gm0 = (cum4.reshape(-1)[ujc] // G) + (gofs_glob - base_jc[ujc])
        starts = np.concatenate([[0], np.cumsum(ucnt)[:-1]])
        rank = np.arange(len(rkey)) - np.repeat(starts, ucnt)
        slot = np.repeat(run_gm0 * G, ucnt) + rank
        eC = np.repeat(ujc % NCHUNK, ucnt)
        idx_streams = [np.full(Spad[c4], PADIDX, np.int16)
                       for c4 in range(NCHUNK)]
        gdloc = [np.full(Spad[c4] // G, -1.0, np.float32)
                 for c4 in range(NCHUNK)]
        gsrc = [np.full(Spad[c4] // G * G, B2 - 1, np.int64)
                for c4 in range(NCHUNK)]
        for c4 in range(NCHUNK):
            mm = eC == c4
            idx_streams[c4][slot[mm]] = (es2[mm] - c4 * C2).astype(np.int16)
            gsrc[c4][slot[mm]] = es2[mm]
        gpos_all = np.repeat(run_gm0, ugr) + (
            np.arange(int(ugr.sum())) - np.repeat(np.cumsum(ugr) - ugr, ugr))
        gval_all = np.repeat(udrow, ugr).astype(np.float32)
        gc4_all = np.repeat(ujc % NCHUNK, ugr)
        for c4 in range(NCHUNK):
            mm = gc4_all == c4
            gdloc[c4][gpos_all[mm]] = gval_all[mm]
        dloc = np.full((P, NENT), -1.0, np.float32)
        for e, (j, c4, K) in enumerate(entries):
            a = int(cum4[j, c4]) // G
            b2 = int(cum4[j, c4] + Q4[j, c4]) // G
            g0 = K * P
            sl = gdloc[c4][g0:g0 + P]
            pos = g0 + np.arange(len(sl))
            dloc[:len(sl), e] = np.where((pos >= a) & (pos < b2), sl, -1.0)
        dv_loc = dinv[orig_of_new[c8 * NPC:(c8 + 1) * NPC]]
        dinv_c = np.zeros((P, TILES), np.float32)
        for j in range(TILES):
            rows = P if j < TILES - 1 else LAST_ROWS
            dinv_c[:rows, j] = dv_loc[j * P:j * P + rows]
        per_core.append(dict(idx=idx_streams, gsrc=gsrc, dloc=dloc,
                             dinvc=dinv_c))
    sched = dict(Q4=Q4, cum4=cum4, S4=S4, CALLS=CALLS, Spad=Spad,
                 entries=entries, NENT=NENT)
    return newid, orig_of_new, dinv, per_core, sched


def _np_reference(x, edge_index, W1, b1, W2, b2, W3, b3):
    src = np.concatenate([edge_index[0].astype(np.int64), np.arange(N)])
    dst = np.concatenate([edge_index[1].astype(np.int64), np.arange(N)])
    deg = np.bincount(dst, minlength=N).astype(np.float32)
    dinv = 1.0 / np.sqrt(deg)

    def agg(v):
        vs = v * dinv[:, None]
        z = np.zeros_like(v)
        np.add.at(z, dst, vs[src])
        return z * dinv[:, None]

    celu = lambda v: np.maximum(v, 0) + np.exp(np.minimum(v, 0)) - 1.0
    h1 = celu(agg(x) @ W1 + b1)
    h2 = celu(agg(h1) @ W2 + b2)
    return celu(agg(h2 @ W3) + b3).astype(np.float32)
